# revision 15
# baseline (speedup 1.0000x reference)
"""Trainium2 Bass kernel for the CAM factorized-attention module.

Reference computation (per batch element b, C=256, N=P*H*W=12288, h=8 heads,
Ch=32):
    x1   = x[b].reshape(C, N).T                      # [N, C]
    qkv  = x1 @ W_qkv + b_qkv                        # [N, 3C]
    q, k, v  (each [h, N, Ch])
    kw   = softmax(k, axis=N)
    kv   = kw^T @ v (per head)                       # [h, Ch, Ch]
    fa   = q @ kv                                    # [h, N, Ch]
    out  = (scale * fa).reshape(N, C) @ W_proj + b_proj
    res  = gamma * out.T.reshape(C, P, H, W) + x[b]

Sharding: data-parallel over B — core i computes batch element i, no
collectives. All matmuls run in bf16 with fp32 PSUM accumulation; the
attention branch is ~0.3% of the output magnitude (output = x + gamma*attn),
so bf16 rounding contributes ~1e-5 relative error end to end.

Algebraic restructuring (exact):
  * k bias cancels in softmax (constant along the softmax axis)  -> dropped.
  * no softmax max-subtraction needed (|k| < ~4); normalization by the
    denominator is applied to the tiny per-head [Ch, Ch] kv matrix, not to
    the [N, C] weight field.  Denominators come for free as an extra ones
    column in the kv matmul.
  * v bias:   kv_true = (E^T v_raw)/S + b_v (row vec)            -> tiny add.
  * scale & gamma fold into W_proj;  gamma folds into b_proj (host side).
  * q is NEVER materialized: fa^T = kvblk^T (Wq^T x + bq 1^T)
        = (Wq_blk kvblk)^T x + (kvblk^T bq) 1^T  =  G^T x + cq 1^T,
    where G = Wq_blk @ kvblk is fused on-chip (4 tiny matmuls) once kv is
    known, and cq is a per-partition bias applied during the PSUM->SBUF copy.

On-chip layouts (per core):
  xbf    [2][128, 12288] bf16  rows = channel c, cols = token n  (resident)
  k||v   per PAIR of 128-token chunks: PSUM [128, 1024] (tokens on parts)
  E      exp(k) bf16 [128, 512] (two chunks batched per ACT op)
  vb     per chunk [128, 258] = [v cols 0..127 | 1 | v cols 128..255 | 1]
  kvps   2x PSUM [128, 129]: E_half^T @ vb_half accumulated over all 96
         chunks; diagonal 32x32 blocks = per-head kv, col 128 = softmax sums
  kvblk  [2][128, 128] bf16 block-diagonal per-head kv (zeros elsewhere) so
         4 heads' fa is ONE full-array matmul (off-diag contributes 0)
"""

import sys

sys.path.insert(0, "/opt/trn_rl_repo")

import numpy as np
import ml_dtypes

import concourse.bacc as bacc
import concourse.mybir as mybir
from concourse.tile import TileContext
from concourse.bass_utils import run_bass_kernel_spmd

FP32 = mybir.dt.float32
BF16 = mybir.dt.bfloat16
AF = mybir.ActivationFunctionType

C = 256
N = 12288
NCORES = 8
NPAIR = N // 256  # 48 pairs of 128-token chunks
NJUMBO = N // 512  # 24 chunks of 512 tokens

_CACHE = {}


def _build_nc(debug=False):
    from concourse.alu_op_type import AluOpType

    nc = bacc.Bacc(trn_type="TRN2", target_bir_lowering=False)

    xbf_d = nc.declare_dram_parameter("xbf", [2, 128, N], BF16, False)
    xf_d = nc.declare_dram_parameter("xf", [2, 128, N], FP32, False)
    wqt_d = nc.declare_dram_parameter("wqt", [2, 128, 256], BF16, False)
    wkv_d = nc.declare_dram_parameter("wkv", [2, 128, 512], BF16, False)
    wp_d = nc.declare_dram_parameter("wp", [2, 128, 256], BF16, False)
    bq_d = nc.declare_dram_parameter("bq", [2, 128, 1], BF16, False)
    bp_d = nc.declare_dram_parameter("bp", [2, 128, 1], FP32, False)
    bv_d = nc.declare_dram_parameter("bv", [2, 128, 32], FP32, False)
    out_d = nc.declare_dram_parameter("out", [2, 128, N], FP32, True)
    if debug:
        dbg_kvps = nc.declare_dram_parameter("dbg_kvps", [2, 128, 129], FP32, True)
        dbg_kvblk = nc.declare_dram_parameter("dbg_kvblk", [2, 128, 128], BF16, True)
        dbg_G = nc.declare_dram_parameter("dbg_G", [2, 2, 128, 128], BF16, True)
        dbg_cq = nc.declare_dram_parameter("dbg_cq", [2, 128, 1], FP32, True)
        dbg_fsb = nc.declare_dram_parameter("dbg_fsb", [2, 128, 512], BF16, True)

    with TileContext(nc) as tc:
        with (
            tc.tile_pool(name="const", bufs=1) as const,
            tc.tile_pool(name="resident", bufs=1) as resident,
        ):
            # --- resident tensors -------------------------------------------
            xbf = [resident.tile([128, N], BF16, name=f"xbf{t}") for t in range(2)]
            wqt = [const.tile([128, 256], BF16, name=f"wqt{t}") for t in range(2)]
            wkv = [const.tile([128, 512], BF16, name=f"wkv{t}") for t in range(2)]
            wp = [const.tile([128, 256], BF16, name=f"wp{t}") for t in range(2)]
            bq = [const.tile([128, 1], BF16, name=f"bq{t}") for t in range(2)]
            bp = [const.tile([128, 1], FP32, name=f"bp{t}") for t in range(2)]
            bv = [const.tile([128, 32], FP32, name=f"bv{t}") for t in range(2)]
            kvblk = [const.tile([128, 128], BF16, name=f"kvblk{t}") for t in range(2)]
            G = [
                [const.tile([128, 128], BF16, name=f"G{t}{kc}") for kc in range(2)]
                for t in range(2)
            ]
            cq = [const.tile([128, 1], FP32, name=f"cq{t}") for t in range(2)]
            recip = [const.tile([128, 1], FP32, name=f"recip{t}") for t in range(2)]

            for t in range(2):
                nc.sync.dma_start(xbf[t][:], xbf_d[t])
                nc.sync.dma_start(wqt[t][:], wqt_d[t])
                nc.sync.dma_start(wkv[t][:], wkv_d[t])
                nc.sync.dma_start(wp[t][:], wp_d[t])
                nc.sync.dma_start(bq[t][:], bq_d[t])
                nc.sync.dma_start(bp[t][:], bp_d[t])
                nc.sync.dma_start(bv[t][:], bv_d[t])
                nc.vector.memset(kvblk[t][:], 0.0)

            # --- phase 1: qkv (k,v), exp, kv accumulation -------------------
            with (
                tc.tile_pool(name="p1ps", bufs=1, space="PSUM") as p1ps,
                tc.tile_pool(name="kvp_ps", bufs=3, space="PSUM") as kvp_ps,
                tc.tile_pool(name="ework", bufs=3) as ework,
                tc.tile_pool(name="vwork", bufs=6) as vwork,
            ):
                kvps = [
                    p1ps.tile([128, 129], FP32, name=f"kvps{t}") for t in range(2)
                ]

                for pi in range(NPAIR):
                    first, last = pi == 0, pi == NPAIR - 1
                    kvp = kvp_ps.tile([128, 1024], FP32, name="kvp", tag="kvp")
                    for half in range(2):
                        n0 = (pi * 2 + half) * 128
                        f0 = half * 512
                        nc.tensor.matmul(
                            kvp[:, f0 : f0 + 512],
                            lhsT=xbf[0][:, n0 : n0 + 128], rhs=wkv[0][:],
                            start=True, stop=False,
                        )
                        nc.tensor.matmul(
                            kvp[:, f0 : f0 + 512],
                            lhsT=xbf[1][:, n0 : n0 + 128], rhs=wkv[1][:],
                            start=False, stop=True,
                        )
                    # one exp over both chunks' k columns (strided view)
                    E = ework.tile([128, 512], BF16, name="E", tag="E")
                    kvp_v = kvp[:].rearrange("p (s x) -> p s x", x=512)
                    E_v = E[:].rearrange("p (s x) -> p s x", x=256)
                    nc.scalar.activation(E_v, kvp_v[:, :, 0:256], AF.Exp)
                    for half in range(2):
                        f0 = half * 512
                        vb = vwork.tile([128, 258], BF16, name="vb", tag="vb")
                        nc.vector.memset(
                            vb[:].rearrange("p (s x) -> p s x", x=129)[:, :, 128:129],
                            1.0,
                        )
                        nc.vector.tensor_copy(
                            vb[:].rearrange("p (s x) -> p s x", x=129)[:, :, 0:128],
                            kvp[:, f0 + 256 : f0 + 512].rearrange(
                                "p (s x) -> p s x", x=128
                            ),
                        )
                        for t in range(2):
                            nc.tensor.matmul(
                                kvps[t][:],
                                lhsT=E[:, half * 256 + t * 128 : half * 256 + t * 128 + 128],
                                rhs=vb[:, t * 129 : t * 129 + 129],
                                start=(first and half == 0),
                                stop=(last and half == 1),
                                skip_group_check=True,
                            )

                # --- finalize kv: normalize rows, add v bias ----------------
                if debug:
                    for t in range(2):
                        kvcp = ework.tile([128, 129], FP32, name=f"kvcp{t}")
                        nc.vector.tensor_copy(kvcp[:], kvps[t][:])
                        nc.sync.dma_start(dbg_kvps[t], kvcp[:])
                for t in range(2):
                    nc.vector.reciprocal(recip[t][:], kvps[t][:, 128:129])
                    for g in range(4):
                        r0 = g * 32
                        nc.vector.scalar_tensor_tensor(
                            kvblk[t][r0 : r0 + 32, r0 : r0 + 32],
                            kvps[t][r0 : r0 + 32, r0 : r0 + 32],
                            recip[t][r0 : r0 + 32, :],
                            bv[t][r0 : r0 + 32, :],
                            op0=AluOpType.mult,
                            op1=AluOpType.add,
                        )

            # --- interphase: G = Wq_blk @ kvblk, cq = kvblk^T @ bq ----------
            with tc.tile_pool(name="gps", bufs=1, space="PSUM") as gps:
                for t in range(2):
                    cq_ps = gps.tile([128, 1], FP32, name=f"cqps{t}")
                    nc.tensor.matmul(
                        cq_ps[:], lhsT=kvblk[t][:], rhs=bq[t][:],
                        start=True, stop=True,
                    )
                    nc.vector.tensor_copy(cq[t][:], cq_ps[:])
                    for kc in range(2):
                        g_ps = gps.tile([128, 128], FP32, name=f"gps{t}{kc}")
                        nc.tensor.matmul(
                            g_ps[:],
                            lhsT=wqt[t][:, kc * 128 : kc * 128 + 128],
                            rhs=kvblk[t][:],
                            start=True, stop=True,
                        )
                        nc.vector.tensor_copy(G[t][kc][:], g_ps[:])
                if debug:
                    for t in range(2):
                        nc.sync.dma_start(dbg_cq[t], cq[t][:])
                        for kc in range(2):
                            nc.sync.dma_start(dbg_G[t][kc], G[t][kc][:])

            # --- phase 2: fa = G^T x + cq, proj, bias + residual ------------
            with (
                tc.tile_pool(name="fa_ps", bufs=4, space="PSUM") as fa_ps,
                tc.tile_pool(name="pp_ps", bufs=4, space="PSUM") as pp_ps,
                tc.tile_pool(name="p2work", bufs=4) as p2work,
                tc.tile_pool(name="p2out", bufs=3) as p2out,
            ):
                for cj in range(NJUMBO):
                    n0 = cj * 512
                    fsb = []
                    for t in range(2):
                        fap = fa_ps.tile([128, 512], FP32, name="fap", tag="fap")
                        nc.tensor.matmul(
                            fap[:], lhsT=G[t][0][:], rhs=xbf[0][:, n0 : n0 + 512],
                            start=True, stop=False,
                        )
                        nc.tensor.matmul(
                            fap[:], lhsT=G[t][1][:], rhs=xbf[1][:, n0 : n0 + 512],
                            start=False, stop=True,
                        )
                        f = p2work.tile([128, 512], BF16, name="fsb", tag="fsb")
                        nc.scalar.activation(f[:], fap[:], AF.Identity, bias=cq[t][:])
                        fsb.append(f)
                        if debug and cj == 0:
                            nc.sync.dma_start(dbg_fsb[t], f[:])
                    for mt in range(2):
                        pp = pp_ps.tile([128, 512], FP32, name="pp", tag="pp")
                        nc.tensor.matmul(
                            pp[:], lhsT=wp[0][:, mt * 128 : mt * 128 + 128],
                            rhs=fsb[0][:], start=True, stop=False,
                        )
                        nc.tensor.matmul(
                            pp[:], lhsT=wp[1][:, mt * 128 : mt * 128 + 128],
                            rhs=fsb[1][:], start=False, stop=True,
                        )
                        xin = p2out.tile([128, 512], FP32, name="xin", tag="xin")
                        nc.sync.dma_start(xin[:], xf_d[mt, :, n0 : n0 + 512])
                        osb = p2out.tile([128, 512], FP32, name="osb", tag="osb")
                        nc.vector.scalar_tensor_tensor(
                            osb[:], pp[:], bp[mt][:], xin[:],
                            op0=AluOpType.add, op1=AluOpType.add,
                        )
                        nc.sync.dma_start(out_d[mt, :, n0 : n0 + 512], osb[:])
            if debug:
                for t in range(2):
                    nc.sync.dma_start(dbg_kvblk[t], kvblk[t][:])
    nc.finalize()
    return nc


def _get_nc():
    if "nc" not in _CACHE:
        _CACHE["nc"] = _build_nc()
    return _CACHE["nc"]


def _prep_in_maps(x, W_qkv, b_qkv, W_proj, b_proj, gamma):
    bf = ml_dtypes.bfloat16
    scale = 32 ** (-0.5)
    g = float(np.asarray(gamma).reshape(-1)[0])

    WqT = np.ascontiguousarray(
        W_qkv[:, 0:256].T.reshape(2, 128, 256)).astype(bf)
    Wkv = np.ascontiguousarray(
        W_qkv[:, 256:768].reshape(2, 128, 512)).astype(bf)
    Wp = np.ascontiguousarray(
        (W_proj * (scale * g)).reshape(2, 128, 256)).astype(bf)
    bq = np.ascontiguousarray(
        b_qkv[0:256].reshape(2, 128, 1)).astype(bf)
    bp = np.ascontiguousarray(
        (g * b_proj).reshape(2, 128, 1)).astype(np.float32)
    # bv[t][p, cv] = b_qkv[512 + (t*4 + p//32)*32 + cv]
    bv = np.ascontiguousarray(
        np.broadcast_to(
            b_qkv[512:768].reshape(2, 4, 1, 32), (2, 4, 32, 32)
        ).reshape(2, 128, 32)
    ).astype(np.float32)

    in_maps = []
    for b in range(NCORES):
        xb = np.ascontiguousarray(x[b].reshape(C, N))
        in_maps.append(
            {
                "xbf": xb.reshape(2, 128, N).astype(bf),
                "xf": xb.reshape(2, 128, N),
                "wqt": WqT, "wkv": Wkv, "wp": Wp,
                "bq": bq, "bp": bp, "bv": bv,
            }
        )
    return in_maps


def kernel(x, W_qkv, b_qkv, W_proj, b_proj, gamma, _trace=False, _trace_kwargs=None):
    x = np.asarray(x, dtype=np.float32)
    nc = _get_nc()
    in_maps = _prep_in_maps(
        x,
        np.asarray(W_qkv, np.float32),
        np.asarray(b_qkv, np.float32),
        np.asarray(W_proj, np.float32),
        np.asarray(b_proj, np.float32),
        np.asarray(gamma, np.float32),
    )
    kw = {}
    if _trace:
        kw = {"trace": True, **(_trace_kwargs or {})}
    res = run_bass_kernel_spmd(nc, in_maps, list(range(NCORES)), **kw)
    out = np.stack(
        [res.results[b]["out"].reshape(C, 3, 64, 64) for b in range(NCORES)]
    ).astype(np.float32)
    if _trace:
        return out, res
    return out


# revision 26
# speedup vs baseline: 1.6091x; 1.6091x over previous
"""Trainium2 Bass kernel for the CAM factorized-attention module.

Reference computation (per batch element b, C=256, N=P*H*W=12288, h=8 heads,
Ch=32):
    x1   = x[b].reshape(C, N).T                      # [N, C]
    qkv  = x1 @ W_qkv + b_qkv                        # [N, 3C]
    q, k, v  (each [h, N, Ch])
    kw   = softmax(k, axis=N)
    kv   = kw^T @ v (per head)                       # [h, Ch, Ch]
    fa   = q @ kv                                    # [h, N, Ch]
    out  = (scale * fa).reshape(N, C) @ W_proj + b_proj
    res  = gamma * out.T.reshape(C, P, H, W) + x[b]

Sharding: data-parallel over B — core i computes batch element i, no
collectives. All matmuls run in bf16 with fp32 PSUM accumulation; the
attention branch is ~0.3% of the output magnitude (output = x + gamma*attn),
so bf16 rounding contributes ~1e-5 relative error end to end.

Algebraic restructuring (exact up to rounding):
  * k bias cancels in softmax (constant along the softmax axis)  -> dropped.
  * no softmax max-subtraction needed (|k| < ~4); the denominator is applied
    to the tiny per-head [Ch, Ch] kv matrix, not the [N, C] weight field.
    Denominators come free as an extra ones column in the kv matmul.
  * v bias folds into kv:  kv_true = (E^T v_raw)/S + b_v (row vec).
  * scale & gamma fold into W_proj;  gamma folds into b_proj (host side).
  * q is never materialized, and once kv is known the whole branch collapses
    to ONE linear map of x:
        attn^T = M^T x + bias_eff 1^T
        M[kc][mt]  = sum_t  Wq[kc,tblk] @ kvblk[t] @ Wp'[tblk,mtblk]
        bias_eff   = sum_t  Wp'[tblk,mtblk]^T kvblk[t]^T bq[tblk] + bp'
    M ([256,256] total) is fused on-chip with 14 tiny matmuls after the kv
    accumulation finishes.

Per-core pipeline:
  load xf (fp32, resident)  ->  cast to xbf (bf16, resident, ACT)
  phase 1 (96 token-chunks of 128):  k||v = xbf^T Wkv  (PSUM, pairs of
    chunks share a [128,1024] 2-bank tile);  E = exp(k) (one ACT op per
    pair);  vb = [v|1] bf16;  kvps[t] += E_half^T vb_half  (PSUM, FD=129)
  finalize: kv = diag-blocks(kvps)/S + bv  -> block-diagonal kvblk (bf16)
  fold:  G' = kvblk^T Wq^T,  M = G'^T Wp',  bias_eff
  phase 2 (24 token-chunks of 512):  pp = M^T xbf  (PSUM);
    osb = (pp + bias_eff) + xf   (one DVE op);  DMA out
"""

import sys

sys.path.insert(0, "/opt/trn_rl_repo")

import numpy as np
import ml_dtypes

import concourse.bacc as bacc
import concourse.mybir as mybir
from concourse.tile import TileContext
from concourse.bass_utils import run_bass_kernel_spmd

FP32 = mybir.dt.float32
BF16 = mybir.dt.bfloat16
AF = mybir.ActivationFunctionType

C = 256
N = 12288
NCORES = 8
NPAIR = N // 256  # 48 pairs of 128-token chunks
NJUMBO = N // 512  # 24 chunks of 512 tokens
NPIECE = 16  # xf load / cast granularity

_CACHE = {}


def _build_nc(debug=False):
    from concourse.alu_op_type import AluOpType

    nc = bacc.Bacc(trn_type="TRN2", target_bir_lowering=False)

    xf_d = nc.declare_dram_parameter("xf", [2, 128, N], FP32, False)
    wqt_d = nc.declare_dram_parameter("wqt", [2, 128, 256], BF16, False)
    wkv_d = nc.declare_dram_parameter("wkv", [2, 128, 512], BF16, False)
    wp_d = nc.declare_dram_parameter("wp", [2, 128, 256], BF16, False)
    bq_d = nc.declare_dram_parameter("bq", [2, 128, 1], BF16, False)
    bp_d = nc.declare_dram_parameter("bp", [2, 128, 1], FP32, False)
    bv_d = nc.declare_dram_parameter("bv", [2, 128, 32], FP32, False)
    out_d = nc.declare_dram_parameter("out", [2, 128, N], FP32, True)
    if debug:
        dbg_kvps = nc.declare_dram_parameter("dbg_kvps", [2, 128, 129], FP32, True)
        dbg_kvblk = nc.declare_dram_parameter("dbg_kvblk", [2, 128, 128], BF16, True)
        dbg_M = nc.declare_dram_parameter("dbg_M", [2, 2, 128, 128], BF16, True)
        dbg_be = nc.declare_dram_parameter("dbg_be", [2, 128, 1], FP32, True)

    PIECE = N // NPIECE

    with TileContext(nc) as tc:
        with (
            tc.tile_pool(name="const", bufs=1) as const,
            tc.tile_pool(name="resident", bufs=1) as resident,
        ):
            # --- resident tensors -------------------------------------------
            xf = [resident.tile([128, N], FP32, name=f"xf{t}") for t in range(2)]
            xbf = [resident.tile([128, N], BF16, name=f"xbf{t}") for t in range(2)]
            wqt = [const.tile([128, 256], BF16, name=f"wqt{t}") for t in range(2)]
            wkv = [const.tile([128, 512], BF16, name=f"wkv{t}") for t in range(2)]
            wp = [const.tile([128, 256], BF16, name=f"wp{t}") for t in range(2)]
            bq = [const.tile([128, 1], BF16, name=f"bq{t}") for t in range(2)]
            bp = [const.tile([128, 1], FP32, name=f"bp{t}") for t in range(2)]
            bv = [const.tile([128, 32], FP32, name=f"bv{t}") for t in range(2)]
            kvblk = [const.tile([128, 128], BF16, name=f"kvblk{t}") for t in range(2)]
            Gp = [
                [const.tile([128, 128], BF16, name=f"Gp{t}{kc}") for kc in range(2)]
                for t in range(2)
            ]
            M = [
                [const.tile([128, 128], BF16, name=f"M{kc}{mt}") for mt in range(2)]
                for kc in range(2)
            ]
            cq = [const.tile([128, 1], BF16, name=f"cq{t}") for t in range(2)]
            be = [const.tile([128, 1], FP32, name=f"be{mt}") for mt in range(2)]
            recip = [const.tile([128, 1], FP32, name=f"recip{t}") for t in range(2)]
            vb = [const.tile([128, 258], BF16, name=f"vb{j}") for j in range(6)]

            # first x piece + the weights phase 1 needs, before everything else
            for t in range(2):
                nc.sync.dma_start(xf[t][:, 0:PIECE], xf_d[t, :, 0:PIECE])
                nc.gpsimd.tensor_copy(xbf[t][:, 0:PIECE], xf[t][:, 0:PIECE])
            for t in range(2):
                nc.sync.dma_start(wkv[t][:], wkv_d[t])
            for t in range(2):
                nc.sync.dma_start(wqt[t][:], wqt_d[t])
                nc.sync.dma_start(wp[t][:], wp_d[t])
                nc.sync.dma_start(bq[t][:], bq_d[t])
                nc.sync.dma_start(bp[t][:], bp_d[t])
                nc.sync.dma_start(bv[t][:], bv_d[t])
                nc.vector.memset(kvblk[t][:], 0.0)
            for j in range(6):
                nc.vector.memset(
                    vb[j][:].rearrange("p (s x) -> p s x", x=129)[:, :, 128:129], 1.0
                )
            # remaining x pieces, load + bf16 cast so compute starts early
            for i in range(1, NPIECE):
                for t in range(2):
                    nc.sync.dma_start(
                        xf[t][:, i * PIECE : (i + 1) * PIECE],
                        xf_d[t, :, i * PIECE : (i + 1) * PIECE],
                    )
                    nc.gpsimd.tensor_copy(
                        xbf[t][:, i * PIECE : (i + 1) * PIECE],
                        xf[t][:, i * PIECE : (i + 1) * PIECE],
                    )

            # --- phase 1: k||v, exp, kv accumulation ------------------------
            kvsum = const.tile([128, 258], FP32, name="kvsum")
            with (
                tc.tile_pool(name="p1ps", bufs=1, space="PSUM") as p1ps,
                tc.tile_pool(name="kvp_ps", bufs=3, space="PSUM") as kvp_ps,
                tc.tile_pool(name="ework", bufs=4) as ework,
            ):
                # two parity-alternating accumulators (t0 at cols 0:129, t1 at
                # 129:258) so consecutive pairs' kv matmuls are independent
                kvps = [
                    p1ps.tile([128, 258], FP32, name=f"kvps{par}") for par in range(2)
                ]

                for pi in range(NPAIR):
                    par = pi % 2
                    first, last = pi < 2, pi >= NPAIR - 2
                    kvp = kvp_ps.tile([128, 1024], FP32, name="kvp", tag="kvp")
                    for half in range(2):
                        n0 = (pi * 2 + half) * 128
                        f0 = half * 512
                        nc.tensor.matmul(
                            kvp[:, f0 : f0 + 512],
                            lhsT=xbf[0][:, n0 : n0 + 128], rhs=wkv[0][:],
                            start=True, stop=False,
                        )
                        nc.tensor.matmul(
                            kvp[:, f0 : f0 + 512],
                            lhsT=xbf[1][:, n0 : n0 + 128], rhs=wkv[1][:],
                            start=False, stop=True,
                        )
                    # one exp over both chunks' k columns (strided view)
                    E = ework.tile([128, 512], BF16, name="E", tag="E")
                    nc.scalar.activation(
                        E[:].rearrange("p (s x) -> p s x", x=256),
                        kvp[:].rearrange("p (s x) -> p s x", x=512)[:, :, 0:256],
                        AF.Exp,
                    )
                    for half in range(2):
                        f0 = half * 512
                        v = vb[(pi * 2 + half) % 6]
                        vdst = v[:].rearrange("p (s x) -> p s x", x=129)[:, :, 0:128]
                        vsrc = kvp[:, f0 + 256 : f0 + 512].rearrange(
                            "p (s x) -> p s x", x=128
                        )
                        nc.vector.tensor_copy(vdst, vsrc)
                        for t in range(2):
                            nc.tensor.matmul(
                                kvps[par][:, t * 129 : t * 129 + 129],
                                lhsT=E[
                                    :,
                                    half * 256 + t * 128 : half * 256 + t * 128 + 128,
                                ],
                                rhs=v[:, t * 129 : t * 129 + 129],
                                start=(first and half == 0),
                                stop=(last and half == 1),
                                skip_group_check=True,
                            )

                # --- finalize kv: merge parities, normalize, add v bias -----
                nc.vector.tensor_copy(kvsum[:], kvps[0][:])
                nc.vector.tensor_add(kvsum[:], kvsum[:], kvps[1][:])
                if debug:
                    for t in range(2):
                        nc.sync.dma_start(
                            dbg_kvps[t], kvsum[:, t * 129 : t * 129 + 129]
                        )
                for t in range(2):
                    c0 = t * 129
                    nc.vector.reciprocal(recip[t][:], kvsum[:, c0 + 128 : c0 + 129])
                    for g in range(4):
                        r0 = g * 32
                        nc.vector.scalar_tensor_tensor(
                            kvblk[t][r0 : r0 + 32, r0 : r0 + 32],
                            kvsum[r0 : r0 + 32, c0 + r0 : c0 + r0 + 32],
                            recip[t][r0 : r0 + 32, :],
                            bv[t][r0 : r0 + 32, :],
                            op0=AluOpType.mult,
                            op1=AluOpType.add,
                        )

            # --- fold: G' = kvblk^T Wq^T, M = G'^T Wp', bias_eff ------------
            with tc.tile_pool(name="gps", bufs=4, space="PSUM") as gps:
                for t in range(2):
                    cq_ps = gps.tile([128, 1], FP32, name=f"cqps{t}", tag="little")
                    nc.tensor.matmul(
                        cq_ps[:], lhsT=kvblk[t][:], rhs=bq[t][:],
                        start=True, stop=True,
                    )
                    nc.vector.tensor_copy(cq[t][:], cq_ps[:])
                    for kc in range(2):
                        g_ps = gps.tile([128, 128], FP32, name=f"gps{t}{kc}", tag="big")
                        nc.tensor.matmul(
                            g_ps[:],
                            lhsT=kvblk[t][:],
                            rhs=wqt[t][:, kc * 128 : kc * 128 + 128],
                            start=True, stop=True,
                        )
                        nc.vector.tensor_copy(Gp[t][kc][:], g_ps[:])
                for mt in range(2):
                    be_ps = gps.tile([128, 1], FP32, name=f"beps{mt}", tag="little")
                    for t in range(2):
                        nc.tensor.matmul(
                            be_ps[:],
                            lhsT=wp[t][:, mt * 128 : mt * 128 + 128],
                            rhs=cq[t][:],
                            start=(t == 0), stop=(t == 1),
                        )
                    nc.vector.tensor_add(be[mt][:], be_ps[:], bp[mt][:])
                    for kc in range(2):
                        m_ps = gps.tile([128, 128], FP32, name=f"mps{kc}{mt}", tag="big")
                        for t in range(2):
                            nc.tensor.matmul(
                                m_ps[:],
                                lhsT=Gp[t][kc][:],
                                rhs=wp[t][:, mt * 128 : mt * 128 + 128],
                                start=(t == 0), stop=(t == 1),
                            )
                        nc.vector.tensor_copy(M[kc][mt][:], m_ps[:])
                if debug:
                    for kc in range(2):
                        for mt in range(2):
                            nc.sync.dma_start(dbg_M[kc][mt], M[kc][mt][:])
                    for mt in range(2):
                        nc.sync.dma_start(dbg_be[mt], be[mt][:])
                    for t in range(2):
                        nc.sync.dma_start(dbg_kvblk[t], kvblk[t][:])

            # --- phase 2: pp = M^T xbf;  out = pp + bias_eff + xf -----------
            with (
                tc.tile_pool(name="pp_ps", bufs=6, space="PSUM") as pp_ps,
                tc.tile_pool(name="p2out", bufs=10) as p2out,
            ):
                for cj in range(NJUMBO):
                    n0 = cj * 512
                    for mt in range(2):
                        pp = pp_ps.tile([128, 512], FP32, name="pp", tag="pp")
                        nc.tensor.matmul(
                            pp[:], lhsT=M[0][mt][:], rhs=xbf[0][:, n0 : n0 + 512],
                            start=True, stop=False,
                        )
                        nc.tensor.matmul(
                            pp[:], lhsT=M[1][mt][:], rhs=xbf[1][:, n0 : n0 + 512],
                            start=False, stop=True,
                        )
                        osb = p2out.tile([128, 512], FP32, name="osb", tag="osb")
                        if mt == 0:
                            nc.vector.scalar_tensor_tensor(
                                osb[:], pp[:], be[mt][:], xf[mt][:, n0 : n0 + 512],
                                op0=AluOpType.add, op1=AluOpType.add,
                            )
                        else:
                            tmp = p2out.tile(
                                [128, 512], FP32, name="tmp", tag="tmp"
                            )
                            nc.scalar.activation(
                                tmp[:], pp[:], AF.Identity, bias=be[mt][:]
                            )
                            nc.gpsimd.tensor_add(
                                osb[:], tmp[:], xf[mt][:, n0 : n0 + 512]
                            )
                        nc.sync.dma_start(out_d[mt, :, n0 : n0 + 512], osb[:])
    nc.finalize()
    return nc


def _get_nc():
    if "nc" not in _CACHE:
        _CACHE["nc"] = _build_nc()
    return _CACHE["nc"]


def _prep_in_maps(x, W_qkv, b_qkv, W_proj, b_proj, gamma):
    bf = ml_dtypes.bfloat16
    scale = 32 ** (-0.5)
    g = float(np.asarray(gamma).reshape(-1)[0])

    WqT = np.ascontiguousarray(
        W_qkv[:, 0:256].T.reshape(2, 128, 256)).astype(bf)
    Wkv = np.ascontiguousarray(
        W_qkv[:, 256:768].reshape(2, 128, 512)).astype(bf)
    Wp = np.ascontiguousarray(
        (W_proj * (scale * g)).reshape(2, 128, 256)).astype(bf)
    bq = np.ascontiguousarray(
        b_qkv[0:256].reshape(2, 128, 1)).astype(bf)
    bp = np.ascontiguousarray(
        (g * b_proj).reshape(2, 128, 1)).astype(np.float32)
    # bv[t][p, cv] = b_qkv[512 + (t*4 + p//32)*32 + cv]
    bv = np.ascontiguousarray(
        np.broadcast_to(
            b_qkv[512:768].reshape(2, 4, 1, 32), (2, 4, 32, 32)
        ).reshape(2, 128, 32)
    ).astype(np.float32)

    in_maps = []
    for b in range(NCORES):
        xb = np.ascontiguousarray(x[b].reshape(C, N))
        in_maps.append(
            {
                "xf": xb.reshape(2, 128, N),
                "wqt": WqT, "wkv": Wkv, "wp": Wp,
                "bq": bq, "bp": bp, "bv": bv,
            }
        )
    return in_maps


def kernel(x, W_qkv, b_qkv, W_proj, b_proj, gamma, _trace=False, _trace_kwargs=None):
    x = np.asarray(x, dtype=np.float32)
    nc = _get_nc()
    in_maps = _prep_in_maps(
        x,
        np.asarray(W_qkv, np.float32),
        np.asarray(b_qkv, np.float32),
        np.asarray(W_proj, np.float32),
        np.asarray(b_proj, np.float32),
        np.asarray(gamma, np.float32),
    )
    kw = {}
    if _trace:
        kw = {"trace": True, **(_trace_kwargs or {})}
    res = run_bass_kernel_spmd(nc, in_maps, list(range(NCORES)), **kw)
    out = np.stack(
        [res.results[b]["out"].reshape(C, 3, 64, 64) for b in range(NCORES)]
    ).astype(np.float32)
    if _trace:
        return out, res
    return out
